# revision 11
# baseline (speedup 1.0000x reference)
"""Trainium2 Bass kernel for the Attractor recurrence, v2 (col-tiled PE).

Math identical to v1: iterate w -> lrelu(w @ M'') with M'' = lam*(M + 0.5 I)
in bf16, normalize once on the host at the end (lrelu is positively
homogeneous, l2norm scale-invariant, decay baked into M, step-1 decay
subtracted via x^T).

New in v2 -- the PE's 128x32 column-tiling mode runs FOUR matmul streams
concurrently (HW-measured 3.92x): col-tile c handles K-tiles ki%4==c,
accumulating into PSUM partitions 32c..32c+8.  The four partial sums are
folded in f32 by ACT+DVE ops whose PSUM operand sits at a different base
partition (HW-verified legal: the equal-base rule only binds when both
inputs are SBUF), rounded to bf16 exactly once, transposed by the DMA XBAR
([16,128]->[128,16] chunks, zero compute-engine cost), lrelu'd by the ACT
engine on the transposed slab, and AllGathered as before.  The final step
ships the reduced f32 pre-activations; the host lrelus and normalizes in
f64.
"""

import numpy as np
import ml_dtypes

B = 8          # batch
D = 8192       # feature dim
NCORES = 8
DK = D // NCORES       # 1024 columns per core
KT = D // 128          # 64 K-tiles of 128
NIB = 16               # nibbles: groups of 4 consecutive K-tiles
TAU = 16
SLOPE = 0.01
LAM = float(2.0 ** -12)

_BF16 = ml_dtypes.bfloat16

USE_PRELU = True   # sim lacks Prelu; tests can flip to the DVE max() pair
H1A_PRE = 4        # h1 A-rounds emitted before h0 B-rounds
NDUMMY = 10        # HAM-keepalive matmuls at each steady-state step start

_cached = {}


def _build_program(tau=TAU):
    import concourse.bass as bass
    import concourse.mybir as mybir
    import concourse.tile as tile
    from concourse import bacc

    fp32 = mybir.dt.float32
    bf16 = mybir.dt.bfloat16
    fp16 = mybir.dt.float16
    ALU = mybir.AluOpType
    PRELU = mybir.ActivationFunctionType.Prelu
    RG = [list(range(NCORES))]

    nc = bacc.Bacc(
        "TRN2",
        target_bir_lowering=False,
        debug=False,
        num_devices=NCORES,
    )

    # m is host-prelinearized: [group, partition, 4 K-tiles x 1024 cols]
    m_dram = nc.dram_tensor("m", [16, 128, 4 * DK], bf16, kind="ExternalInput")
    xt_dram = nc.dram_tensor("xt", [128, KT * B], bf16, kind="ExternalInput")
    xtsh_dram = nc.dram_tensor("xtsh", [128, 8 * B], bf16, kind="ExternalInput")
    out_dram = nc.dram_tensor("out", [B, 2 * 512], fp32, kind="ExternalOutput")

    EVEN_NIB = [n for n in range(NIB) if n % 2 == 0]   # covered by AG#1
    ODD_NIB = [n for n in range(NIB) if n % 2 == 1]    # covered by AG#2

    with tile.TileContext(nc, num_cores=NCORES) as tc:
        with (
            tc.tile_pool(name="mpool", bufs=1) as mpool,
            tc.tile_pool(name="consts", bufs=1) as consts,
            tc.tile_pool(name="state", bufs=2) as state,
            tc.tile_pool(name="qpool", bufs=4) as qpool,
            tc.tile_pool(name="tpool", bufs=4) as tpool,
            tc.tile_pool(name="wpool", bufs=4) as wpool,
            tc.tile_pool(name="fin", bufs=1) as fin,
            tc.tile_pool(name="mmps", bufs=3, space="PSUM") as mmps,
            tc.tile_pool(name="dps", bufs=1, space="PSUM") as dps,
            tc.tile_pool(name="dram", bufs=8, space="DRAM") as dram,
        ):
            # warm-up AllGathers: absorb first-collective staging while M loads
            for w in range(2):
                warm_in = dram.tile([128 * 8 * B], bf16, tag="wag_in",
                                    name=f"warmi{w}")
                warm_out = dram.tile([NCORES * 128 * 8 * B], bf16,
                                     tag="wag_out", name=f"warmo{w}")
                nc.sync.dma_start(
                    out=warm_in.rearrange("(p c) -> p c", p=128),
                    in_=xt_dram.ap()[:, : 8 * B],
                )
                nc.gpsimd.collective_compute(
                    "AllGather", ALU.bypass, replica_groups=RG,
                    ins=[warm_in[:]], outs=[warm_out[:]],
                )

            # static XBAR staging tiles; rows 8:16 stay zero forever
            q16s = {}
            for hh in range(2):
                q16s[hh] = consts.tile([16, 512], bf16, tag=f"q16_{hh}",
                                       name=f"q16_{hh}")
                nc.vector.memzero(q16s[hh][:])

            # small constants ahead of the bulk M load
            xt_sb = consts.tile([128, KT * B], bf16)
            nc.sync.dma_start(out=xt_sb[:], in_=xt_dram.ap())
            xtsh_sb = consts.tile([128, 8 * B], bf16)
            nc.sync.dma_start(out=xtsh_sb[:], in_=xtsh_dram.ap())

            # resident M'' shard: 16 groups of 4 K-tiles, contiguous loads
            m_tiles = {}
            load_engines = [nc.sync, nc.scalar, nc.gpsimd]
            for g in range(16):
                mt = mpool.tile([128, 4 * DK], bf16, tag=f"m{g}")
                load_engines[g % len(load_engines)].dma_start(
                    out=mt[:], in_=m_dram.ap()[g]
                )
                m_tiles[g] = mt

            def m_ap(ki, col0, ncol):
                g, kk = divmod(ki, 4)
                return m_tiles[g][:, kk * DK + col0 : kk * DK + col0 + ncol]

            cur_vT = xt_sb  # step-1 stationary operand = bf16(x)^T

            def mm_round(ps, n, h, start, stop):
                """One col-tiled round: 4 concurrent MMs, K-tiles 4n+c."""
                for c in range(4):
                    ki = 4 * n + c
                    nc.tensor.matmul(
                        ps[32 * c : 32 * c + B, :],
                        cur_vT[:, ki * B : (ki + 1) * B],
                        m_ap(ki, h * 512, 512),
                        start=start, stop=stop,
                        tile_position=(0, 32 * c),
                    )

            def dummies(t, n):
                dp = dps.tile([128, 512], fp32, tag="dps", name=f"dps{t}")
                for _ in range(n):
                    nc.tensor.matmul(
                        dp[0:B, :], xt_sb[:, 0:B], m_tiles[0][:, 0:512],
                        start=True, stop=True, tile_position=(0, 0),
                    )

            def half_post(t, h, ps, w_T):
                """4-group psum partials -> f32 folds (one bf16 rounding) ->
                XBAR transpose -> lrelu'd [128, 32] bf16 slab for AllGather."""
                q32 = qpool.tile([B, 512], fp32, tag="q32", name=f"q{t}_{h}")
                nc.scalar.copy(out=q32[:], in_=ps[0:B, :])
                nc.vector.tensor_tensor(
                    out=q32[:], in0=q32[:], in1=ps[32 : 32 + B, :],
                    op=mybir.AluOpType.add,
                )
                nc.vector.tensor_tensor(
                    out=q32[:], in0=q32[:], in1=ps[64 : 64 + B, :],
                    op=mybir.AluOpType.add,
                )
                q16 = q16s[h]
                nc.vector.tensor_tensor(
                    out=q16[0:B, :], in0=q32[:], in1=ps[96 : 96 + B, :],
                    op=mybir.AluOpType.add,
                )
                trq = tpool.tile([128, 4 * 16], bf16, tag="trq",
                                 name=f"tr{t}_{h}")
                eng = nc.sync if h == 0 else nc.scalar
                eng.dma_start(
                    out=trq.rearrange("p (c j) -> p c j", c=4),
                    in_=q16[:],
                    transpose=True,
                )
                v = trq.rearrange("p (c s) -> p c s", c=4)
                red = v[:, :, 0:B]
                if t == 0:
                    # step 1 has hs=0: remove the baked 0.5*lam decay term
                    xsl = xtsh_sb.rearrange("p (hh c b) -> p hh c b", hh=2, c=4)
                    nc.vector.scalar_tensor_tensor(
                        out=red, in0=xsl[:, h], scalar=-0.5 * LAM,
                        in1=red, op0=mybir.AluOpType.mult,
                        op1=mybir.AluOpType.add,
                    )
                wv = w_T[:, h * 4 * B : (h + 1) * 4 * B].rearrange(
                    "p (c b) -> p c b", c=4)
                if USE_PRELU:
                    nc.scalar.activation(out=wv[:], in_=red, func=PRELU,
                                         alpha=SLOPE)
                else:
                    a = tpool.tile([128, 4 * B], bf16, tag="lr",
                                   name=f"lr{t}_{h}")
                    av = a.rearrange("p (c b) -> p c b", c=4)
                    nc.vector.tensor_scalar_mul(av[:], red, SLOPE)
                    nc.vector.tensor_tensor(out=wv[:], in0=red, in1=av[:],
                                            op=mybir.AluOpType.max)

            def step_gather(t, w_T):
                """One AllGather for the full [128, 64] step slab."""
                ag_in = dram.tile([128 * 8 * B], bf16, tag="ag_in",
                                  name=f"agi{t}")
                ag_out = dram.tile([NCORES * 128 * 8 * B], bf16,
                                   tag="ag_out", name=f"ago{t}")
                nc.scalar.dma_start(
                    out=ag_in.rearrange("(p c) -> p c", p=128), in_=w_T[:]
                )
                nc.gpsimd.collective_compute(
                    "AllGather", ALU.bypass, replica_groups=RG,
                    ins=[ag_in[:]], outs=[ag_out[:]],
                )
                return ag_out

            def step_scatter(ag_out, nxt_vT):
                """ag_out rank blocks -> nxt_vT cols [r*64, +64)."""
                dst = nxt_vT[:].rearrange("p (r c) -> p r c", c=8 * B)
                src = ag_out.rearrange("(r p c) -> p r c", p=128, c=8 * B)
                nc.sync.dma_start(out=dst[:, 0:2], in_=src[:, 0:2])
                nc.sync.dma_start(out=dst[:, 2:5], in_=src[:, 2:5])
                nc.sync.dma_start(out=dst[:, 5:8], in_=src[:, 5:8])

            for t in range(tau):
                last = t == tau - 1
                ps = [
                    mmps.tile([128, 512], fp32, tag="ps", name=f"ps{t}_{h}")
                    for h in range(2)
                ]
                nxt_vT = None if last else state.tile([128, KT * B], bf16)

                if last:
                    # reduced f32 pre-activations -> host lrelu+normalize
                    for n in range(NIB):
                        mm_round(ps[0], n, 0, n == 0, n == NIB - 1)
                    for n in range(NIB):
                        mm_round(ps[1], n, 1, n == 0, n == NIB - 1)
                    o_f = fin.tile([B, 2 * 512], fp32)
                    for h in range(2):
                        osl = o_f[:, h * 512 : (h + 1) * 512]
                        nc.scalar.copy(out=osl, in_=ps[h][0:B, :])
                        for gb in (32, 64, 96):
                            nc.vector.tensor_tensor(
                                out=osl, in0=osl,
                                in1=ps[h][gb : gb + B, :],
                                op=mybir.AluOpType.add,
                            )
                    nc.sync.dma_start(out=out_dram.ap(), in_=o_f[:])
                    continue

                w_T = wpool.tile([128, 8 * B], bf16, tag="wT",
                                 name=f"wT{t}")
                if t == 0:
                    # chase the M load group by group
                    for g in range(16):
                        mm_round(ps[0], g, 0, g == 0, g == 15)
                    half_post(t, 0, ps[0], w_T)
                    for g in range(16):
                        mm_round(ps[1], g, 1, g == 0, g == 15)
                    half_post(t, 1, ps[1], w_T)
                else:
                    dummies(t, NDUMMY)
                    for n in range(NIB):
                        mm_round(ps[0], n, 0, n == 0, n == NIB - 1)
                    half_post(t, 0, ps[0], w_T)
                    for n in range(NIB):
                        mm_round(ps[1], n, 1, n == 0, n == NIB - 1)
                    half_post(t, 1, ps[1], w_T)
                ago = step_gather(t, w_T)
                step_scatter(ago, nxt_vT)

                cur_vT = nxt_vT

    nc.finalize()
    return nc


def _get_program(tau=TAU):
    key = (tau, USE_PRELU, H1A_PRE, NDUMMY)
    if key not in _cached:
        _cached[key] = _build_program(tau)
    return _cached[key]


def _prep_inputs(x, M):
    """Host-side shard prep. Returns list of 8 per-core input dicts."""
    lam = np.float32(LAM)
    xt = (
        x.reshape(B, KT, 128)
        .transpose(2, 1, 0)  # [128, KT, B]
        .reshape(128, KT * B)
        .astype(_BF16)
    )
    in_maps = []
    idx = np.arange(DK)
    for r in range(NCORES):
        cols = slice(r * DK, (r + 1) * DK)
        m_shard = M[:, cols] * lam
        m_shard[r * DK + idx, idx] += np.float32(0.5) * lam
        m_lin = np.ascontiguousarray(
            m_shard.astype(_BF16)
            .reshape(16, 4, 128, DK)
            .transpose(0, 2, 1, 3)
            .reshape(16, 128, 4 * DK)
        )
        in_maps.append(
            {
                "m": m_lin,
                "xt": xt,
                "xtsh": np.ascontiguousarray(xt[:, r * 8 * B : (r + 1) * 8 * B]),
            }
        )
    return in_maps


def _finish(res):
    """lrelu the reduced pre-activations, concat shards, normalize (f64)."""
    shards = []
    for r in range(NCORES):
        pre = res.results[r]["out"].astype(np.float64)  # [8, 1024]
        shards.append(np.where(pre >= 0, pre, SLOPE * pre))
    v = np.concatenate(shards, axis=1)  # [8, 8192]
    nrm = np.sqrt((v ** 2).sum(axis=1, keepdims=True))
    return (v / nrm).astype(np.float32)


def kernel(x, M, hs):
    from concourse.bass_utils import run_bass_kernel_spmd

    x = np.asarray(x, dtype=np.float32)
    M = np.asarray(M, dtype=np.float32)
    nc = _get_program()
    in_maps = _prep_inputs(x, M)
    res = run_bass_kernel_spmd(nc, in_maps, core_ids=list(range(NCORES)))
    return _finish(res)
